# revision 35
# baseline (speedup 1.0000x reference)
"""Trainium2 Bass kernel for a gated-delta-rule decode step (B=1, T=1).

Tensor-parallel over 8 NeuronCores by value-head groups:
  - per core: 2 key heads (DK=128), 4 value heads (DV=128)
  - row-shards of in_proj_{qkv,z,b,a}_w / conv_state / conv_w,
    column-shard of out_proj_w, head-shard of rec_state
  - out_proj partials are summed on the host (8 x 2048 floats).

All GEMVs run on the tensor engine: the host ships W^T, the hidden
vector is the stationary operand ([128,1] per K-chunk), and the weights
stream through as the moving operand, accumulating [1, N<=512] PSUM rows
across 16 K-chunks. The weights therefore stream HBM -> SBUF -> PE once,
with fully contiguous DMAs, and the vector/scalar engines stay nearly
idle (conv, gating, per-head delta-rule rows only).
"""

import numpy as np

import bass_rust
import concourse.bass as bass
import concourse.tile as tile
from concourse import mybir
from concourse.bass_utils import run_bass_kernel_spmd
from concourse.vector_clock import ScopedClock


def _patched_drain_and_barrier(self, tick_clock, wait_clock):
    """Tail drain with at most ONE sem wait per instruction.

    The stock TileContext attaches every outstanding semaphore wait to a
    single Drain; this walrus build's CTRL encoding only has room for one
    sync-wait, so codegen dies with "Too many sync wait commands". Split
    the extra waits onto dedicated NOPs on the same (SP) engine.
    """
    nc = self.nc
    drain_inst = nc.sync.drain()
    wait_clock.add_sem_waits(
        drain_inst.ins, ScopedClock({None: tick_clock.global_clock})
    )
    si = drain_inst.ins.sync_info
    if si is not None and si.on_wait is not None and len(si.on_wait) > 1:
        waits = list(si.on_wait)
        si.on_wait = waits[:1]
        for w in waits[1:]:
            nop = nc.sync.nop(nofuse=True)
            nop.ins.sync_info = bass_rust.SyncInfo(on_wait=[w], on_update=[])
    nc.all_engine_barrier()
    assert self.sems is not None
    popped = nc._tile_sem_poison_stack.pop()
    assert popped is self._sem_poison
    _clear_and_free_semaphores_no_rangeclear(
        nc, list(self.sems.allocated().values()))
    nc.all_engine_barrier()


def _clear_and_free_semaphores_no_rangeclear(nc, sems):
    """clear_and_free_semaphores minus EVENT_SEMAPHORE_RANGE_CLEAR.

    That raw-ISA opcode's 64B encoding is rejected by this walrus build
    ("ISA wrong length"); zero each semaphore with a plain EventSemaphore
    sem-wr-imm update instead, spread across engines.
    """
    if not sems:
        return
    sem_nums = [s.num if isinstance(s, bass.SemaphoreHandle) else s for s in sems]
    for sem_range in bass.compact_to_ranges(sem_nums):
        assert nc._state.free_isdisjoint(sem_range)
        nc.gpsimd.dma_reset(sem_range)
    engines = [nc.gpsimd, nc.vector, nc.scalar, nc.tensor, nc.sync]
    for i, s in enumerate(sems):
        eng = engines[i % len(engines)]
        ev = eng.sem_inc(s, 1)
        u = ev.ins.sync_info.on_update[0]
        ev.ins.sync_info = bass_rust.SyncInfo(
            on_wait=[],
            on_update=[bass_rust.SyncUpdate(
                sync_type='semaphore', id=u.id, ant_name=u.ant_name,
                update_mode='sem-wr-imm', update_value=0, update_reg=None)],
        )
    nc._state.prepend_free_semaphores(sem_nums)
    for poison_set in nc._tile_sem_poison_stack:
        poison_set.update(sem_nums)


tile.TileContext._drain_and_barrier = _patched_drain_and_barrier


def _split_multi_waits(nc):
    """Hoist extra semaphore waits onto same-engine NOPs.

    This walrus build's codegen accepts at most ONE sync-wait per
    instruction ("Too many sync wait commands"); Tile's wait-assignment
    attaches up to three. A NOP on the same engine immediately before the
    instruction is semantically identical (engines execute in order).
    """
    n = 0
    for func in nc.m.functions:
        for blk in func.blocks:
            out = []
            changed = False
            for ins in blk.instructions:
                si = ins.sync_info
                if si is not None and si.on_wait is not None and len(si.on_wait) > 1:
                    waits = list(si.on_wait)
                    for w in waits[:-1]:
                        n += 1
                        nop = mybir.InstNoOp(
                            name=f"WSPLIT-{n}", engine=ins.engine,
                            bass_nofuse=True,
                            sync_info=bass_rust.SyncInfo(on_wait=[w], on_update=[]),
                        )
                        nc.register_instruction(nop, overwrite=True)
                        out.append(nop)
                    si.on_wait = waits[-1:]   # in place: keep fake updates
                    changed = True
                out.append(ins)
            if changed:
                blk.instructions = out


F32 = mybir.dt.float32
AF = mybir.ActivationFunctionType
OP = mybir.AluOpType

# ---- problem constants (hardcoded; kernel.py must be self-contained) ----
H = 2048
NUM_K = 16
NUM_V = 32
DK = 128
DV = 128
KSZ = 4
KEY_DIM = NUM_K * DK            # 2048
VALUE_DIM = NUM_V * DV          # 4096
CONV_DIM = 2 * KEY_DIM + VALUE_DIM  # 8192
EPS = 1e-6
NCORES = 8

# per-core shard sizes
NKH = NUM_K // NCORES           # 2 key heads / core
NVH = NUM_V // NCORES           # 4 value heads / core
QROWS = NKH * DK                # 256
VROWS = NVH * DV                # 512
CLOC = 2 * QROWS + VROWS        # 1024 local conv rows
NCT = CLOC // 128               # 8 local conv tiles
CTOT = CLOC + VROWS + 2 * NVH   # 1544 = qkv + z + b + a
NCH = H // 128                  # 16 K-chunks of the hidden dim
P = 128

_CACHE: dict = {}
LAST_RESULTS = None


def _build_nc(sim_compat: bool = False) -> bass.Bass:
    nc = bass.Bass(trn_type="TRN2")

    # ---- per-core DRAM tensors (kernel-friendly layouts; host permutes) ----
    hvec = nc.dram_tensor("hvec", [H], F32, kind="ExternalInput")
    # W^T of [qkv_w; z_w; b_w; a_w] -> (H, 1544)
    w_t = nc.dram_tensor("w_t", [H, CTOT], F32, kind="ExternalInput")
    # conv state/weights pre-permuted on host to [p, t, w] (c = t*128 + p)
    conv_st = nc.dram_tensor("conv_st", [P, NCT, KSZ], F32, kind="ExternalInput")
    conv_wt = nc.dram_tensor("conv_wt", [P, NCT, KSZ], F32, kind="ExternalInput")
    # rec pre-permuted on host to [k, h, v]
    rec_in = nc.dram_tensor("rec_in", [DK, NVH, DV], F32, kind="ExternalInput")
    # out_proj column-shard, transposed: (512, 2048)
    op_t = nc.dram_tensor("op_t", [VROWS, H], F32, kind="ExternalInput")
    dt_b = nc.dram_tensor("dt_b", [NVH], F32, kind="ExternalInput")
    a_log = nc.dram_tensor("a_log", [NVH], F32, kind="ExternalInput")
    norm_w = nc.dram_tensor("norm_w", [DV], F32, kind="ExternalInput")

    hid_out = nc.dram_tensor("hid_out", [H], F32, kind="ExternalOutput")
    conv_out = nc.dram_tensor("conv_out", [P, NCT, KSZ], F32, kind="ExternalOutput")
    rec_out = nc.dram_tensor("rec_out", [DK, NVH, DV], F32, kind="ExternalOutput")

    # DRAM scratch: broadcast g_exp to all partitions (rec_new scaling),
    # plus row->column re-layouts (SBUF->SBUF transposing DMAs don't balance)
    gex_dram = nc.dram_tensor("gex_scratch", [NVH], F32)
    qk_dram = nc.dram_tensor("qk_scratch", [CLOC], F32)
    outv_dram = nc.dram_tensor("outv_scratch", [VROWS], F32)

    with tile.TileContext(nc) as tc:
        with (
            tc.tile_pool(name="singles", bufs=1) as singles,
            tc.tile_pool(name="wpool", bufs=3) as wpool,
            tc.tile_pool(name="oppool", bufs=4) as oppool,
            tc.tile_pool(name="rows", bufs=4) as rows,
            tc.tile_pool(name="psy", bufs=1, space="PSUM") as psy,
            tc.tile_pool(name="psh", bufs=1, space="PSUM") as psh,
        ):
            # x as columns: xcol[p, c] = h[c*128 + p]
            xcol = singles.tile([P, NCH], F32)
            nc.sync.dma_start(out=xcol[:], in_=hvec.rearrange("(c p) -> p c", p=P))

            # ---- combined projection y = W_all @ h on the PE ----
            # psum rows: y[0:512], y[512:1024], y[1024:1536] (z), y[1536:1544] (b,a)
            py = [psy.tile([1, 512], F32, tag=f"py{i}", name=f"py{i}")
                  for i in range(3)]
            pba = psy.tile([1, 2 * NVH], F32, tag="pba")
            for kd in range(NCH // 2):
                wt = wpool.tile([P, 2, CTOT], F32, tag="w")
                nc.sync.dma_start(
                    out=wt[:],
                    in_=w_t[256 * kd:256 * (kd + 1), :].rearrange(
                        "(a p) c -> p a c", p=P),
                )
                for a in range(2):
                    kk = 2 * kd + a
                    st = dict(start=(kk == 0), stop=(kk == NCH - 1))
                    lhs = xcol[:, kk:kk + 1]
                    for i in range(3):
                        nc.tensor.matmul(
                            py[i][:], lhs, wt[:, a, 512 * i:512 * (i + 1)], **st)
                    nc.tensor.matmul(pba[:], lhs, wt[:, a, 1536:CTOT], **st)

            # drain psums to an SBUF row y_row[0, 0:1544]
            y_row = singles.tile([1, CTOT], F32)
            nc.vector.tensor_copy(out=y_row[:, 0:512], in_=py[0][:])
            nc.vector.tensor_copy(out=y_row[:, 512:1024], in_=py[1][:])
            nc.scalar.copy(out=y_row[:, 1024:1536], in_=py[2][:])
            nc.scalar.copy(out=y_row[:, 1536:CTOT], in_=pba[:])
            z_row = y_row[:, 1024:1536]

            # mixed_qkv back to columns via DRAM: qcols[p, t] = y[t*128 + p]
            nc.sync.dma_start(out=qk_dram[:], in_=y_row[0:1, 0:CLOC])
            qcols = singles.tile([P, NCT], F32)
            nc.sync.dma_start(out=qcols[:],
                              in_=qk_dram.rearrange("(t p) -> p t", p=P))

            # ---- causal conv1d update ----
            cs = singles.tile([P, NCT, KSZ], F32)
            cw = singles.tile([P, NCT, KSZ], F32)
            nc.sync.dma_start(out=cs[:], in_=conv_st[:, :, :])
            nc.sync.dma_start(out=cw[:], in_=conv_wt[:, :, :])
            ncs = singles.tile([P, NCT, KSZ], F32)
            nc.vector.tensor_copy(out=ncs[:, :, 0:KSZ - 1], in_=cs[:, :, 1:KSZ])
            nc.vector.tensor_copy(out=ncs[:, :, KSZ - 1:KSZ], in_=qcols[:, :, None])
            nc.sync.dma_start(out=conv_out[:, :, :], in_=ncs[:])

            convp = singles.tile([P, NCT, KSZ], F32)
            nc.vector.tensor_tensor(convp[:], ncs[:], cw[:], OP.mult)
            convpre = singles.tile([P, NCT], F32)
            nc.vector.tensor_reduce(
                out=convpre[:], in_=convp[:], axis=mybir.AxisListType.X, op=OP.add,
            )
            # conv_z_cols columns: 0-1 = k heads, 2-3 = q heads, 4-7 = v heads
            # silu(x) = x / (1 + exp(-x)); only exp/ln ACT tables exist here
            conv_z_cols = singles.tile([P, 32], F32)
            nc.vector.memset(conv_z_cols[:], 0.0)
            sgt = singles.tile([P, NCT], F32)
            nc.scalar.activation(out=sgt[:], in_=convpre[:], func=AF.Exp, scale=-1.0)
            nc.vector.tensor_scalar_add(out=sgt[:], in0=sgt[:], scalar1=1.0)
            nc.vector.reciprocal(out=sgt[:], in_=sgt[:])
            silu_all = singles.tile([P, NCT], F32)
            nc.vector.tensor_tensor(silu_all[:], convpre[:], sgt[:], OP.mult)
            nc.vector.tensor_copy(out=conv_z_cols[:, 2:4], in_=silu_all[:, 0:2])
            nc.vector.tensor_copy(out=conv_z_cols[:, 0:2], in_=silu_all[:, 2:4])
            nc.vector.tensor_copy(out=conv_z_cols[:, 4:NCT], in_=silu_all[:, 4:NCT])

            # ---- transpose -> rows_t[32, 128]; rows 0-1 k, 2-3 q, 4-7 v ----
            rows_t = singles.tile([32, P], F32)
            for b in range(4):
                nc.vector.transpose(
                    out=rows_t[:, 32 * b:32 * (b + 1)],
                    in_=conv_z_cols[32 * b:32 * (b + 1), 0:32],
                )
            # head rows onto partition 0 (engines need base 0/32/64/96)
            rows0 = singles.tile([1, NCT, P], F32)
            nc.sync.dma_start(out=rows0[:], in_=rows_t[0:NCT, None, :])

            # ---- per-head scalars, all on partition 0 ----
            # ss: sum of squares of k/q rows -> ss_row[0, 0:4] (k0,k1,q0,q1)
            sqr = singles.tile([1, 4, P], F32)
            nc.vector.tensor_tensor(sqr[:], rows0[:, 0:4, :], rows0[:, 0:4, :],
                                    OP.mult)
            ss_row = singles.tile([1, 4], F32)
            nc.vector.tensor_reduce(
                out=ss_row[:], in_=sqr[:], axis=mybir.AxisListType.X, op=OP.add)

            eps1 = singles.tile([1, 1], F32)
            nc.vector.memset(eps1[:], EPS)
            epsk1 = singles.tile([1, 1], F32)
            nc.vector.memset(epsk1[:], float(DK) * EPS)
            # rv_row: cols 0-1 rq_eff(g) = 1/sqrt(128*(ss_q+eps)),
            #         cols 2-3 rk(g)     = 1/sqrt(ss_k+eps)
            l_row = singles.tile([1, 4], F32)
            nc.scalar.activation(out=l_row[:, 0:2], in_=ss_row[:, 2:4], func=AF.Ln,
                                 bias=epsk1[:], scale=float(DK))
            nc.scalar.activation(out=l_row[:, 2:4], in_=ss_row[:, 0:2], func=AF.Ln,
                                 bias=eps1[:], scale=1.0)
            rv_row = singles.tile([1, 4], F32)
            nc.scalar.activation(out=rv_row[:], in_=l_row[:], func=AF.Exp,
                                 scale=-0.5)

            # gating from ba row: b = y[1536:1540], a = y[1540:1544]
            b_row = y_row[:, 1536:1540]
            a_row = y_row[:, 1540:1544]
            dt_row = singles.tile([1, NVH], F32)
            nc.sync.dma_start(out=dt_row[:], in_=dt_b[None, :])
            al_row = singles.tile([1, NVH], F32)
            nc.sync.dma_start(out=al_row[:], in_=a_log[None, :])
            norm_row = singles.tile([1, DV], F32)
            nc.sync.dma_start(out=norm_row[:], in_=norm_w[None, :])

            beta_row = singles.tile([1, NVH], F32)
            nc.scalar.activation(out=beta_row[:], in_=b_row, func=AF.Exp, scale=-1.0)
            nc.vector.tensor_scalar_add(out=beta_row[:], in0=beta_row[:], scalar1=1.0)
            nc.vector.reciprocal(out=beta_row[:], in_=beta_row[:])

            t4a = singles.tile([1, NVH], F32)
            nc.vector.tensor_tensor(t4a[:], a_row, dt_row[:], OP.add)
            sp4 = singles.tile([1, NVH], F32)
            nc.scalar.activation(out=sp4[:], in_=t4a[:], func=AF.Exp)
            nc.scalar.activation(out=sp4[:], in_=sp4[:], func=AF.Ln, bias=1.0)
            ea4 = singles.tile([1, NVH], F32)
            nc.scalar.activation(out=ea4[:], in_=al_row[:], func=AF.Exp)
            t4b = singles.tile([1, NVH], F32)
            nc.vector.tensor_tensor(t4b[:], ea4[:], sp4[:], OP.mult)
            gexp_row = singles.tile([1, NVH], F32)
            nc.scalar.activation(out=gexp_row[:], in_=t4b[:], func=AF.Exp, scale=-1.0)

            # broadcast g_exp to all partitions via DRAM (for rec_new scaling)
            nc.sync.dma_start(out=gex_dram[:], in_=gexp_row[:])
            gexp_b = singles.tile([P, NVH], F32)
            gex_src = bass.AP(
                tensor=gex_dram[:].tensor, offset=gex_dram[:].offset,
                ap=[[0, P]] + list(gex_dram[:].ap),
            )
            nc.sync.dma_start(out=gexp_b[:], in_=gex_src)

            # cg = g_exp * rk(g); nbr = -beta * rk(g)   (per head, partition 0)
            rk_rep = rv_row[:, 2:4, None].to_broadcast((1, 2, 2))
            cg_row = singles.tile([1, NVH], F32)
            nc.vector.tensor_tensor(
                cg_row[:].rearrange("o (a b) -> o a b", a=2),
                gexp_row[:].rearrange("o (a b) -> o a b", a=2), rk_rep, OP.mult)
            nbr_row = singles.tile([1, NVH], F32)
            nc.vector.scalar_tensor_tensor(
                out=nbr_row[:].rearrange("o (a b) -> o a b", a=2),
                in0=beta_row[:].rearrange("o (a b) -> o a b", a=2),
                scalar=-1.0, in1=rk_rep, op0=OP.mult, op1=OP.mult)

            # ---- recurrent state update per head ----
            rec_sb = singles.tile([DK, NVH, DV], F32)
            nc.sync.dma_start(out=rec_sb[:], in_=rec_in[:, :, :])
            rec_new = singles.tile([DK, NVH, DV], F32)
            out_cat = singles.tile([1, VROWS], F32)

            for h in range(NVH):
                g = h // 2
                qcol = conv_z_cols[:, 2 + g:3 + g]
                kcol = conv_z_cols[:, g:g + 1]
                krow = rows0[:, g, :]
                vrow = rows0[:, 4 + h, :]
                zrow = z_row[:, h * DV:(h + 1) * DV]
                rec_h = rec_sb[:, h, :]

                # m = rec . k_raw
                psum_m = psh.tile([1, DV], F32, tag="pm")
                nc.tensor.matmul(psum_m[:], kcol, rec_h, start=True, stop=True)
                # delta_eff = ((g_exp*rk)*m - v) * (-beta*rk)
                delta1 = rows.tile([1, DV], F32, tag="d1")
                nc.vector.scalar_tensor_tensor(
                    out=delta1[:], in0=psum_m[:], scalar=cg_row[:, h:h + 1],
                    in1=vrow, op0=OP.mult, op1=OP.subtract)
                delta_eff = rows.tile([1, DV], F32, tag="de")
                nc.vector.tensor_scalar_mul(
                    out=delta_eff[:], in0=delta1[:], scalar1=nbr_row[:, h:h + 1])
                # rank-1 update + decay
                psum_rec = psh.tile([DK, DV], F32, tag="pr")
                nc.tensor.matmul(psum_rec[:], krow, delta_eff[:], start=True,
                                 stop=True)
                nc.vector.scalar_tensor_tensor(
                    out=rec_new[:, h, :], in0=rec_h, scalar=gexp_b[:, h:h + 1],
                    in1=psum_rec[:], op0=OP.mult, op1=OP.add)
                # core = rq_eff * (rec_new . q_raw)
                psum_c = psh.tile([1, DV], F32, tag="pc")
                nc.tensor.matmul(psum_c[:], qcol, rec_new[:, h, :], start=True,
                                 stop=True)
                core_row = rows.tile([1, DV], F32, tag="cr")
                nc.vector.tensor_scalar_mul(
                    out=core_row[:], in0=psum_c[:], scalar1=rv_row[:, g:g + 1])

                # RMS norm + silu(z) gate
                sq1 = rows.tile([1, DV], F32, tag="sq1")
                nc.vector.tensor_tensor(sq1[:], core_row[:], core_row[:], OP.mult)
                var1 = rows.tile([1, 1], F32, tag="var")
                nc.vector.tensor_reduce(
                    out=var1[:], in_=sq1[:], axis=mybir.AxisListType.X, op=OP.add)
                sd1 = rows.tile([1, 1], F32, tag="sd")
                nc.scalar.activation(out=sd1[:], in_=var1[:], func=AF.Ln,
                                     bias=eps1[:], scale=1.0 / DV)
                rstd = rows.tile([1, 1], F32, tag="rstd")
                nc.scalar.activation(out=rstd[:], in_=sd1[:], func=AF.Exp,
                                     scale=-0.5)
                siluz = rows.tile([1, DV], F32, tag="sz")
                nc.scalar.activation(out=siluz[:], in_=zrow, func=AF.Exp, scale=-1.0)
                nc.vector.tensor_scalar_add(out=siluz[:], in0=siluz[:], scalar1=1.0)
                nc.vector.reciprocal(out=siluz[:], in_=siluz[:])
                nc.vector.tensor_tensor(siluz[:], siluz[:], zrow, OP.mult)
                xn = rows.tile([1, DV], F32, tag="xn")
                nc.vector.scalar_tensor_tensor(
                    out=xn[:], in0=core_row[:], scalar=rstd[:],
                    in1=norm_row[:], op0=OP.mult, op1=OP.mult)
                nc.vector.tensor_tensor(
                    out_cat[:, h * DV:(h + 1) * DV], xn[:], siluz[:], OP.mult)

            nc.sync.dma_start(out=rec_out[:, :, :], in_=rec_new[:])

            # out as columns via DRAM: out_col[p, h] = out_cat[0, h*128 + p]
            nc.sync.dma_start(out=outv_dram[:], in_=out_cat[:])
            out_col = singles.tile([P, NVH], F32)
            nc.sync.dma_start(out=out_col[:],
                              in_=outv_dram.rearrange("(t p) -> p t", p=P))

            # ---- out_proj partial on the PE: hid = op_w[:, shard] @ out ----
            opts = []
            for v in range(NVH):
                opt = oppool.tile([P, H], F32, tag="opw")
                nc.sync.dma_start(out=opt[:], in_=op_t[v * P:(v + 1) * P, :])
                opts.append(opt)
            # reuse the projection psum banks (drained long ago)
            ph = [psy.tile([1, 512], F32, tag=(f"py{i}" if i < 3 else "pba"),
                           name=f"ph{i}") for i in range(4)]
            for v in range(NVH):
                for i in range(4):
                    nc.tensor.matmul(
                        ph[i][:], out_col[:, v:v + 1],
                        opts[v][:, 512 * i:512 * (i + 1)],
                        start=(v == 0), stop=(v == NVH - 1))
            hid_row = singles.tile([1, H], F32)
            nc.vector.tensor_copy(out=hid_row[:, 0:512], in_=ph[0][:])
            nc.vector.tensor_copy(out=hid_row[:, 512:1024], in_=ph[1][:])
            nc.scalar.copy(out=hid_row[:, 1024:1536], in_=ph[2][:])
            nc.scalar.copy(out=hid_row[:, 1536:2048], in_=ph[3][:])
            nc.sync.dma_start(out=hid_out[:], in_=hid_row[:])

    _split_multi_waits(nc)
    return nc


def _shard_inputs(inputs: dict) -> list[dict]:
    """Slice the full inputs into 8 per-core input maps (kernel layouts)."""
    hidden_in = np.ascontiguousarray(inputs["hidden_in"], dtype=np.float32)
    conv_state = np.ascontiguousarray(inputs["conv_state"], dtype=np.float32)
    rec_state = np.ascontiguousarray(inputs["rec_state"], dtype=np.float32)
    conv_w = np.ascontiguousarray(inputs["conv_w"], dtype=np.float32)
    qkv_w = np.ascontiguousarray(inputs["in_proj_qkv_w"], dtype=np.float32)
    z_w = np.ascontiguousarray(inputs["in_proj_z_w"], dtype=np.float32)
    b_w = np.ascontiguousarray(inputs["in_proj_b_w"], dtype=np.float32)
    a_w = np.ascontiguousarray(inputs["in_proj_a_w"], dtype=np.float32)
    op_w = np.ascontiguousarray(inputs["out_proj_w"], dtype=np.float32)
    dt_bias = np.ascontiguousarray(inputs["dt_bias"], dtype=np.float32)
    a_log = np.ascontiguousarray(inputs["A_log"], dtype=np.float32)
    norm_w = np.ascontiguousarray(inputs["norm_w"], dtype=np.float32)

    hvec = hidden_in.reshape(H)
    in_maps = []
    for i in range(NCORES):
        qsl = slice(QROWS * i, QROWS * (i + 1))
        ksl = slice(KEY_DIM + QROWS * i, KEY_DIM + QROWS * (i + 1))
        vsl = slice(2 * KEY_DIM + VROWS * i, 2 * KEY_DIM + VROWS * (i + 1))
        hsl = slice(NVH * i, NVH * (i + 1))
        zsl = slice(VROWS * i, VROWS * (i + 1))

        def crows(x):
            return np.concatenate([x[qsl], x[ksl], x[vsl]], axis=0)

        w_all = np.concatenate(
            [crows(qkv_w), z_w[zsl], b_w[hsl], a_w[hsl]], axis=0)  # (1544, H)
        cst = crows(conv_state[0]).reshape(NCT, P, KSZ).transpose(1, 0, 2)
        cwt = crows(conv_w).reshape(NCT, P, KSZ).transpose(1, 0, 2)
        rec_i = rec_state[0, hsl].transpose(1, 0, 2)  # (k, h, v)

        in_maps.append({
            "hvec": np.ascontiguousarray(hvec),
            "w_t": np.ascontiguousarray(w_all.T),
            "conv_st": np.ascontiguousarray(cst),
            "conv_wt": np.ascontiguousarray(cwt),
            "rec_in": np.ascontiguousarray(rec_i),
            "op_t": np.ascontiguousarray(op_w[:, zsl].T),
            "dt_b": np.ascontiguousarray(dt_bias[hsl]),
            "a_log": np.ascontiguousarray(a_log[hsl]),
            "norm_w": np.ascontiguousarray(norm_w),
        })
    return in_maps


def _gather_outputs(results: list[dict]) -> tuple:
    hidden = np.zeros(H, dtype=np.float64)
    conv_full = np.zeros((CONV_DIM, KSZ), dtype=np.float32)
    rec_full = np.zeros((NUM_V, DK, DV), dtype=np.float32)
    for i, r in enumerate(results):
        hidden += r["hid_out"].astype(np.float64)
        ncs = r["conv_out"].transpose(1, 0, 2).reshape(CLOC, KSZ)
        conv_full[QROWS * i:QROWS * (i + 1)] = ncs[:QROWS]
        conv_full[KEY_DIM + QROWS * i:KEY_DIM + QROWS * (i + 1)] = ncs[QROWS:2 * QROWS]
        conv_full[2 * KEY_DIM + VROWS * i:2 * KEY_DIM + VROWS * (i + 1)] = ncs[2 * QROWS:]
        rec_full[NVH * i:NVH * (i + 1)] = r["rec_out"].transpose(1, 0, 2)
    return (
        hidden.astype(np.float32).reshape(1, 1, H),
        conv_full.reshape(1, CONV_DIM, KSZ),
        rec_full.reshape(1, NUM_V, DK, DV),
    )


def kernel(**inputs):
    global LAST_RESULTS
    import os
    nc = _CACHE.get("nc")
    if nc is None:
        nc = _build_nc()
        _CACHE["nc"] = nc
    in_maps = _shard_inputs(inputs)
    kwargs = {}
    if os.environ.get("KERNEL_TRACE"):
        kwargs = dict(trace=True, tmpdir=os.environ.get("KERNEL_TRACE_DIR"))
    res = run_bass_kernel_spmd(nc, in_maps, core_ids=list(range(NCORES)), **kwargs)
    LAST_RESULTS = res
    return _gather_outputs(res.results)


# revision 37
# speedup vs baseline: 1.0313x; 1.0313x over previous
"""Trainium2 Bass kernel for a gated-delta-rule decode step (B=1, T=1).

Tensor-parallel over 8 NeuronCores by value-head groups:
  - per core: 2 key heads (DK=128), 4 value heads (DV=128)
  - row-shards of in_proj_{qkv,z,b,a}_w / conv_state / conv_w,
    column-shard of out_proj_w, head-shard of rec_state
  - out_proj partials are summed on the host (8 x 2048 floats).

All GEMVs run on the tensor engine: the host ships W^T, the hidden
vector is the stationary operand ([128,1] per K-chunk), and the weights
stream through as the moving operand, accumulating [1, N<=512] PSUM rows
across 16 K-chunks. The weights therefore stream HBM -> SBUF -> PE once,
with fully contiguous DMAs, and the vector/scalar engines stay nearly
idle (conv, gating, per-head delta-rule rows only).
"""

import numpy as np

import bass_rust
import concourse.bass as bass
import concourse.tile as tile
from concourse import mybir
from concourse.bass_utils import run_bass_kernel_spmd
from concourse.vector_clock import ScopedClock


def _patched_drain_and_barrier(self, tick_clock, wait_clock):
    """Tail drain with at most ONE sem wait per instruction.

    The stock TileContext attaches every outstanding semaphore wait to a
    single Drain; this walrus build's CTRL encoding only has room for one
    sync-wait, so codegen dies with "Too many sync wait commands". Split
    the extra waits onto dedicated NOPs on the same (SP) engine.
    """
    nc = self.nc
    drain_inst = nc.sync.drain()
    wait_clock.add_sem_waits(
        drain_inst.ins, ScopedClock({None: tick_clock.global_clock})
    )
    si = drain_inst.ins.sync_info
    if si is not None and si.on_wait is not None and len(si.on_wait) > 1:
        waits = list(si.on_wait)
        si.on_wait = waits[:1]
        for w in waits[1:]:
            nop = nc.sync.nop(nofuse=True)
            nop.ins.sync_info = bass_rust.SyncInfo(on_wait=[w], on_update=[])
    nc.all_engine_barrier()
    assert self.sems is not None
    popped = nc._tile_sem_poison_stack.pop()
    assert popped is self._sem_poison
    _clear_and_free_semaphores_no_rangeclear(
        nc, list(self.sems.allocated().values()))
    nc.all_engine_barrier()


def _clear_and_free_semaphores_no_rangeclear(nc, sems):
    """clear_and_free_semaphores minus EVENT_SEMAPHORE_RANGE_CLEAR.

    That raw-ISA opcode's 64B encoding is rejected by this walrus build
    ("ISA wrong length"); zero each semaphore with a plain EventSemaphore
    sem-wr-imm update instead, spread across engines.
    """
    if not sems:
        return
    sem_nums = [s.num if isinstance(s, bass.SemaphoreHandle) else s for s in sems]
    for sem_range in bass.compact_to_ranges(sem_nums):
        assert nc._state.free_isdisjoint(sem_range)
        nc.gpsimd.dma_reset(sem_range)
    engines = [nc.gpsimd, nc.vector, nc.scalar, nc.tensor, nc.sync]
    for i, s in enumerate(sems):
        eng = engines[i % len(engines)]
        ev = eng.sem_inc(s, 1)
        u = ev.ins.sync_info.on_update[0]
        ev.ins.sync_info = bass_rust.SyncInfo(
            on_wait=[],
            on_update=[bass_rust.SyncUpdate(
                sync_type='semaphore', id=u.id, ant_name=u.ant_name,
                update_mode='sem-wr-imm', update_value=0, update_reg=None)],
        )
    nc._state.prepend_free_semaphores(sem_nums)
    for poison_set in nc._tile_sem_poison_stack:
        poison_set.update(sem_nums)


tile.TileContext._drain_and_barrier = _patched_drain_and_barrier


def _split_multi_waits(nc):
    """Hoist extra semaphore waits onto same-engine NOPs.

    This walrus build's codegen accepts at most ONE sync-wait per
    instruction ("Too many sync wait commands"); Tile's wait-assignment
    attaches up to three. A NOP on the same engine immediately before the
    instruction is semantically identical (engines execute in order).
    """
    n = 0
    for func in nc.m.functions:
        for blk in func.blocks:
            out = []
            changed = False
            for ins in blk.instructions:
                si = ins.sync_info
                if si is not None and si.on_wait is not None and len(si.on_wait) > 1:
                    waits = list(si.on_wait)
                    for w in waits[:-1]:
                        n += 1
                        nop = mybir.InstNoOp(
                            name=f"WSPLIT-{n}", engine=ins.engine,
                            bass_nofuse=True,
                            sync_info=bass_rust.SyncInfo(on_wait=[w], on_update=[]),
                        )
                        nc.register_instruction(nop, overwrite=True)
                        out.append(nop)
                    si.on_wait = waits[-1:]   # in place: keep fake updates
                    changed = True
                out.append(ins)
            if changed:
                blk.instructions = out


F32 = mybir.dt.float32
AF = mybir.ActivationFunctionType
OP = mybir.AluOpType

# ---- problem constants (hardcoded; kernel.py must be self-contained) ----
H = 2048
NUM_K = 16
NUM_V = 32
DK = 128
DV = 128
KSZ = 4
KEY_DIM = NUM_K * DK            # 2048
VALUE_DIM = NUM_V * DV          # 4096
CONV_DIM = 2 * KEY_DIM + VALUE_DIM  # 8192
EPS = 1e-6
NCORES = 8

# per-core shard sizes
NKH = NUM_K // NCORES           # 2 key heads / core
NVH = NUM_V // NCORES           # 4 value heads / core
QROWS = NKH * DK                # 256
VROWS = NVH * DV                # 512
CLOC = 2 * QROWS + VROWS        # 1024 local conv rows
NCT = CLOC // 128               # 8 local conv tiles
CTOT = CLOC + VROWS + 2 * NVH   # 1544 = qkv + z + b + a
NCH = H // 128                  # 16 K-chunks of the hidden dim
P = 128

_CACHE: dict = {}
LAST_RESULTS = None


def _build_nc(sim_compat: bool = False) -> bass.Bass:
    nc = bass.Bass(trn_type="TRN2")

    # ---- per-core DRAM tensors (kernel-friendly layouts; host permutes) ----
    hvec = nc.dram_tensor("hvec", [H], F32, kind="ExternalInput")
    # qkv rows for the DVE+ACT GEMV path: (1024, H)
    qkv_w = nc.dram_tensor("qkv_w", [CLOC, H], F32, kind="ExternalInput")
    # W^T of [z_w; b_w; a_w] for the PE GEMV path -> (H, 520)
    wpe_t = nc.dram_tensor("wpe_t", [H, VROWS + 2 * NVH], F32,
                           kind="ExternalInput")
    # conv state/weights pre-permuted on host to [p, t, w] (c = t*128 + p)
    conv_st = nc.dram_tensor("conv_st", [P, NCT, KSZ], F32, kind="ExternalInput")
    conv_wt = nc.dram_tensor("conv_wt", [P, NCT, KSZ], F32, kind="ExternalInput")
    # rec pre-permuted on host to [k, h, v]
    rec_in = nc.dram_tensor("rec_in", [DK, NVH, DV], F32, kind="ExternalInput")
    # out_proj column-shard: h[0:1024] transposed for PE, h[1024:2048] rows
    op_t = nc.dram_tensor("op_t", [VROWS, H // 2], F32, kind="ExternalInput")
    op_rows = nc.dram_tensor("op_rows", [H // 2, VROWS], F32,
                             kind="ExternalInput")
    dt_b = nc.dram_tensor("dt_b", [NVH], F32, kind="ExternalInput")
    a_log = nc.dram_tensor("a_log", [NVH], F32, kind="ExternalInput")
    norm_w = nc.dram_tensor("norm_w", [DV], F32, kind="ExternalInput")

    hid_row_out = nc.dram_tensor("hid_row_out", [H // 2], F32,
                                 kind="ExternalOutput")
    hid_cols_out = nc.dram_tensor("hid_cols_out", [P, H // 2 // P], F32,
                                  kind="ExternalOutput")
    conv_out = nc.dram_tensor("conv_out", [P, NCT, KSZ], F32, kind="ExternalOutput")
    rec_out = nc.dram_tensor("rec_out", [DK, NVH, DV], F32, kind="ExternalOutput")

    # DRAM scratch: broadcast g_exp to all partitions (rec_new scaling),
    # plus row->column re-layouts (SBUF->SBUF transposing DMAs don't balance)
    gex_dram = nc.dram_tensor("gex_scratch", [NVH], F32)
    outv_dram = nc.dram_tensor("outv_scratch", [VROWS], F32)

    with tile.TileContext(nc) as tc:
        with (
            tc.tile_pool(name="singles", bufs=1) as singles,
            tc.tile_pool(name="wpool", bufs=2) as wpool,
            tc.tile_pool(name="prods", bufs=2) as prods,
            tc.tile_pool(name="oppool", bufs=2) as oppool,
            tc.tile_pool(name="rows", bufs=4) as rows,
            tc.tile_pool(name="psy", bufs=1, space="PSUM") as psy,
            tc.tile_pool(name="psh", bufs=1, space="PSUM") as psh,
        ):
            # x as columns (PE stationary operand): xcol[p, c] = h[c*128 + p]
            xcol = singles.tile([P, NCH], F32)
            nc.sync.dma_start(out=xcol[:], in_=hvec.rearrange("(c p) -> p c", p=P))
            # x broadcast to all partitions (DVE GEMV operand)
            xb = singles.tile([P, H], F32)
            hvec_b = bass.AP(
                tensor=hvec[:].tensor, offset=hvec[:].offset,
                ap=[[0, P]] + list(hvec[:].ap),
            )
            nc.sync.dma_start(out=xb[:], in_=hvec_b)

            # ---- qkv projection on DVE (multiply) + ACT (free-axis reduce) --
            # accumulators land directly as conv columns: qcols[p, t]
            qcols = singles.tile([P, NCT], F32)
            for td in range(NCT // 2):
                wt = wpool.tile([P, 2, H], F32, tag="w")
                nc.sync.dma_start(
                    out=wt[:],
                    in_=qkv_w[256 * td:256 * (td + 1), :].rearrange(
                        "(a p) c -> p a c", p=P),
                )
                prod = prods.tile([P, 2, H], F32, tag="prod")
                nc.vector.tensor_tensor(
                    prod[:], wt[:], xb[:, None, :].to_broadcast((P, 2, H)),
                    OP.mult)
                for a in range(2):
                    nc.scalar.activation(
                        out=prod[:, a, :], in_=prod[:, a, :], func=AF.Copy,
                        accum_out=qcols[:, 2 * td + a:2 * td + a + 1])

            # ---- z/b/a projection on the PE (psum rows) ----
            ZBA = VROWS + 2 * NVH
            pz = psy.tile([1, VROWS], F32, tag="pz")
            pba = psy.tile([1, 2 * NVH], F32, tag="pba")
            for kd in range(NCH // 4):
                wt2 = wpool.tile([P, 4, ZBA], F32, tag="wpe")
                nc.sync.dma_start(
                    out=wt2[:],
                    in_=wpe_t[512 * kd:512 * (kd + 1), :].rearrange(
                        "(a p) c -> p a c", p=P),
                )
                for a in range(4):
                    kk = 4 * kd + a
                    st = dict(start=(kk == 0), stop=(kk == NCH - 1))
                    lhs = xcol[:, kk:kk + 1]
                    nc.tensor.matmul(pz[:], lhs, wt2[:, a, 0:VROWS], **st)
                    nc.tensor.matmul(pba[:], lhs, wt2[:, a, VROWS:ZBA], **st)

            # drain psums to SBUF rows
            zba_row = singles.tile([1, ZBA], F32)
            nc.vector.tensor_copy(out=zba_row[:, 0:VROWS], in_=pz[:])
            nc.vector.tensor_copy(out=zba_row[:, VROWS:ZBA], in_=pba[:])
            z_row = zba_row[:, 0:VROWS]

            # ---- causal conv1d update ----
            cs = singles.tile([P, NCT, KSZ], F32)
            cw = singles.tile([P, NCT, KSZ], F32)
            nc.sync.dma_start(out=cs[:], in_=conv_st[:, :, :])
            nc.sync.dma_start(out=cw[:], in_=conv_wt[:, :, :])
            ncs = singles.tile([P, NCT, KSZ], F32)
            nc.vector.tensor_copy(out=ncs[:, :, 0:KSZ - 1], in_=cs[:, :, 1:KSZ])
            nc.vector.tensor_copy(out=ncs[:, :, KSZ - 1:KSZ], in_=qcols[:, :, None])
            nc.sync.dma_start(out=conv_out[:, :, :], in_=ncs[:])

            convp = singles.tile([P, NCT, KSZ], F32)
            nc.vector.tensor_tensor(convp[:], ncs[:], cw[:], OP.mult)
            convpre = singles.tile([P, NCT], F32)
            nc.vector.tensor_reduce(
                out=convpre[:], in_=convp[:], axis=mybir.AxisListType.X, op=OP.add,
            )
            # conv_z_cols columns: 0-1 = k heads, 2-3 = q heads, 4-7 = v heads
            # silu(x) = x / (1 + exp(-x)); only exp/ln ACT tables exist here
            conv_z_cols = singles.tile([P, 32], F32)
            nc.vector.memset(conv_z_cols[:], 0.0)
            sgt = singles.tile([P, NCT], F32)
            nc.scalar.activation(out=sgt[:], in_=convpre[:], func=AF.Exp, scale=-1.0)
            nc.vector.tensor_scalar_add(out=sgt[:], in0=sgt[:], scalar1=1.0)
            nc.vector.reciprocal(out=sgt[:], in_=sgt[:])
            silu_all = singles.tile([P, NCT], F32)
            nc.vector.tensor_tensor(silu_all[:], convpre[:], sgt[:], OP.mult)
            nc.vector.tensor_copy(out=conv_z_cols[:, 2:4], in_=silu_all[:, 0:2])
            nc.vector.tensor_copy(out=conv_z_cols[:, 0:2], in_=silu_all[:, 2:4])
            nc.vector.tensor_copy(out=conv_z_cols[:, 4:NCT], in_=silu_all[:, 4:NCT])

            # ---- transpose -> rows_t[32, 128]; rows 0-1 k, 2-3 q, 4-7 v ----
            rows_t = singles.tile([32, P], F32)
            for b in range(4):
                nc.vector.transpose(
                    out=rows_t[:, 32 * b:32 * (b + 1)],
                    in_=conv_z_cols[32 * b:32 * (b + 1), 0:32],
                )
            # head rows onto partition 0 (engines need base 0/32/64/96)
            rows0 = singles.tile([1, NCT, P], F32)
            nc.sync.dma_start(out=rows0[:], in_=rows_t[0:NCT, None, :])

            # ---- per-head scalars, all on partition 0 ----
            # ss: sum of squares of k/q rows -> ss_row[0, 0:4] (k0,k1,q0,q1)
            sqr = singles.tile([1, 4, P], F32)
            nc.vector.tensor_tensor(sqr[:], rows0[:, 0:4, :], rows0[:, 0:4, :],
                                    OP.mult)
            ss_row = singles.tile([1, 4], F32)
            nc.vector.tensor_reduce(
                out=ss_row[:], in_=sqr[:], axis=mybir.AxisListType.X, op=OP.add)

            eps1 = singles.tile([1, 1], F32)
            nc.vector.memset(eps1[:], EPS)
            epsk1 = singles.tile([1, 1], F32)
            nc.vector.memset(epsk1[:], float(DK) * EPS)
            # rv_row: cols 0-1 rq_eff(g) = 1/sqrt(128*(ss_q+eps)),
            #         cols 2-3 rk(g)     = 1/sqrt(ss_k+eps)
            l_row = singles.tile([1, 4], F32)
            nc.scalar.activation(out=l_row[:, 0:2], in_=ss_row[:, 2:4], func=AF.Ln,
                                 bias=epsk1[:], scale=float(DK))
            nc.scalar.activation(out=l_row[:, 2:4], in_=ss_row[:, 0:2], func=AF.Ln,
                                 bias=eps1[:], scale=1.0)
            rv_row = singles.tile([1, 4], F32)
            nc.scalar.activation(out=rv_row[:], in_=l_row[:], func=AF.Exp,
                                 scale=-0.5)

            # gating from ba row
            b_row = zba_row[:, VROWS:VROWS + NVH]
            a_row = zba_row[:, VROWS + NVH:ZBA]
            dt_row = singles.tile([1, NVH], F32)
            nc.sync.dma_start(out=dt_row[:], in_=dt_b[None, :])
            al_row = singles.tile([1, NVH], F32)
            nc.sync.dma_start(out=al_row[:], in_=a_log[None, :])
            norm_row = singles.tile([1, DV], F32)
            nc.sync.dma_start(out=norm_row[:], in_=norm_w[None, :])

            beta_row = singles.tile([1, NVH], F32)
            nc.scalar.activation(out=beta_row[:], in_=b_row, func=AF.Exp, scale=-1.0)
            nc.vector.tensor_scalar_add(out=beta_row[:], in0=beta_row[:], scalar1=1.0)
            nc.vector.reciprocal(out=beta_row[:], in_=beta_row[:])

            t4a = singles.tile([1, NVH], F32)
            nc.vector.tensor_tensor(t4a[:], a_row, dt_row[:], OP.add)
            sp4 = singles.tile([1, NVH], F32)
            nc.scalar.activation(out=sp4[:], in_=t4a[:], func=AF.Exp)
            nc.scalar.activation(out=sp4[:], in_=sp4[:], func=AF.Ln, bias=1.0)
            ea4 = singles.tile([1, NVH], F32)
            nc.scalar.activation(out=ea4[:], in_=al_row[:], func=AF.Exp)
            t4b = singles.tile([1, NVH], F32)
            nc.vector.tensor_tensor(t4b[:], ea4[:], sp4[:], OP.mult)
            gexp_row = singles.tile([1, NVH], F32)
            nc.scalar.activation(out=gexp_row[:], in_=t4b[:], func=AF.Exp, scale=-1.0)

            # broadcast g_exp to all partitions via DRAM (for rec_new scaling)
            nc.sync.dma_start(out=gex_dram[:], in_=gexp_row[:])
            gexp_b = singles.tile([P, NVH], F32)
            gex_src = bass.AP(
                tensor=gex_dram[:].tensor, offset=gex_dram[:].offset,
                ap=[[0, P]] + list(gex_dram[:].ap),
            )
            nc.sync.dma_start(out=gexp_b[:], in_=gex_src)

            # cg = g_exp * rk(g); nbr = -beta * rk(g)   (per head, partition 0)
            rk_rep = rv_row[:, 2:4, None].to_broadcast((1, 2, 2))
            cg_row = singles.tile([1, NVH], F32)
            nc.vector.tensor_tensor(
                cg_row[:].rearrange("o (a b) -> o a b", a=2),
                gexp_row[:].rearrange("o (a b) -> o a b", a=2), rk_rep, OP.mult)
            nbr_row = singles.tile([1, NVH], F32)
            nc.vector.scalar_tensor_tensor(
                out=nbr_row[:].rearrange("o (a b) -> o a b", a=2),
                in0=beta_row[:].rearrange("o (a b) -> o a b", a=2),
                scalar=-1.0, in1=rk_rep, op0=OP.mult, op1=OP.mult)

            # ---- recurrent state update per head ----
            rec_sb = singles.tile([DK, NVH, DV], F32)
            nc.sync.dma_start(out=rec_sb[:], in_=rec_in[:, :, :])
            rec_new = singles.tile([DK, NVH, DV], F32)
            out_cat = singles.tile([1, VROWS], F32)

            for h in range(NVH):
                g = h // 2
                qcol = conv_z_cols[:, 2 + g:3 + g]
                kcol = conv_z_cols[:, g:g + 1]
                krow = rows0[:, g, :]
                vrow = rows0[:, 4 + h, :]
                zrow = z_row[:, h * DV:(h + 1) * DV]
                rec_h = rec_sb[:, h, :]

                # m = rec . k_raw
                psum_m = psh.tile([1, DV], F32, tag="pm")
                nc.tensor.matmul(psum_m[:], kcol, rec_h, start=True, stop=True)
                # delta_eff = ((g_exp*rk)*m - v) * (-beta*rk)
                delta1 = rows.tile([1, DV], F32, tag="d1")
                nc.vector.scalar_tensor_tensor(
                    out=delta1[:], in0=psum_m[:], scalar=cg_row[:, h:h + 1],
                    in1=vrow, op0=OP.mult, op1=OP.subtract)
                delta_eff = rows.tile([1, DV], F32, tag="de")
                nc.vector.tensor_scalar_mul(
                    out=delta_eff[:], in0=delta1[:], scalar1=nbr_row[:, h:h + 1])
                # rank-1 update + decay
                psum_rec = psh.tile([DK, DV], F32, tag="pr")
                nc.tensor.matmul(psum_rec[:], krow, delta_eff[:], start=True,
                                 stop=True)
                nc.vector.scalar_tensor_tensor(
                    out=rec_new[:, h, :], in0=rec_h, scalar=gexp_b[:, h:h + 1],
                    in1=psum_rec[:], op0=OP.mult, op1=OP.add)
                # core = rq_eff * (rec_new . q_raw)
                psum_c = psh.tile([1, DV], F32, tag="pc")
                nc.tensor.matmul(psum_c[:], qcol, rec_new[:, h, :], start=True,
                                 stop=True)
                core_row = rows.tile([1, DV], F32, tag="cr")
                nc.vector.tensor_scalar_mul(
                    out=core_row[:], in0=psum_c[:], scalar1=rv_row[:, g:g + 1])

                # RMS norm + silu(z) gate
                sq1 = rows.tile([1, DV], F32, tag="sq1")
                nc.vector.tensor_tensor(sq1[:], core_row[:], core_row[:], OP.mult)
                var1 = rows.tile([1, 1], F32, tag="var")
                nc.vector.tensor_reduce(
                    out=var1[:], in_=sq1[:], axis=mybir.AxisListType.X, op=OP.add)
                sd1 = rows.tile([1, 1], F32, tag="sd")
                nc.scalar.activation(out=sd1[:], in_=var1[:], func=AF.Ln,
                                     bias=eps1[:], scale=1.0 / DV)
                rstd = rows.tile([1, 1], F32, tag="rstd")
                nc.scalar.activation(out=rstd[:], in_=sd1[:], func=AF.Exp,
                                     scale=-0.5)
                siluz = rows.tile([1, DV], F32, tag="sz")
                nc.scalar.activation(out=siluz[:], in_=zrow, func=AF.Exp, scale=-1.0)
                nc.vector.tensor_scalar_add(out=siluz[:], in0=siluz[:], scalar1=1.0)
                nc.vector.reciprocal(out=siluz[:], in_=siluz[:])
                nc.vector.tensor_tensor(siluz[:], siluz[:], zrow, OP.mult)
                xn = rows.tile([1, DV], F32, tag="xn")
                nc.vector.scalar_tensor_tensor(
                    out=xn[:], in0=core_row[:], scalar=rstd[:],
                    in1=norm_row[:], op0=OP.mult, op1=OP.mult)
                nc.vector.tensor_tensor(
                    out_cat[:, h * DV:(h + 1) * DV], xn[:], siluz[:], OP.mult)

            nc.sync.dma_start(out=rec_out[:, :, :], in_=rec_new[:])

            # out as columns via DRAM: out_col[p, h] = out_cat[0, h*128 + p]
            nc.sync.dma_start(out=outv_dram[:], in_=out_cat[:])
            out_col = singles.tile([P, NVH], F32)
            nc.sync.dma_start(out=out_col[:],
                              in_=outv_dram.rearrange("(t p) -> p t", p=P))

            # out broadcast to all partitions (DVE op-proj operand)
            out_b = singles.tile([P, VROWS], F32)
            outv_b = bass.AP(
                tensor=outv_dram[:].tensor, offset=outv_dram[:].offset,
                ap=[[0, P]] + list(outv_dram[:].ap),
            )
            nc.sync.dma_start(out=out_b[:], in_=outv_b)

            # ---- out_proj h[0:1024] on the PE ----
            opt = oppool.tile([P, NVH, H // 2], F32, tag="opw")
            nc.sync.dma_start(
                out=opt[:],
                in_=op_t[:, :].rearrange("(v p) c -> p v c", p=P))
            ph = [psy.tile([1, 512], F32, tag=("pz" if i == 0 else "pba"),
                           name=f"ph{i}") for i in range(2)]
            for v in range(NVH):
                for i in range(2):
                    nc.tensor.matmul(
                        ph[i][:], out_col[:, v:v + 1],
                        opt[:, v, 512 * i:512 * (i + 1)],
                        start=(v == 0), stop=(v == NVH - 1))
            hid_row = singles.tile([1, H // 2], F32)
            nc.vector.tensor_copy(out=hid_row[:, 0:512], in_=ph[0][:])
            nc.vector.tensor_copy(out=hid_row[:, 512:1024], in_=ph[1][:])
            nc.sync.dma_start(out=hid_row_out[:], in_=hid_row[:])

            # ---- out_proj h[1024:2048] on DVE+ACT (column accumulators) ----
            hid_cols = singles.tile([P, H // 2 // P], F32)
            for td in range(2):
                opr = oppool.tile([P, 4, VROWS], F32, tag="oprow")
                nc.sync.dma_start(
                    out=opr[:],
                    in_=op_rows[512 * td:512 * (td + 1), :].rearrange(
                        "(a p) c -> p a c", p=P))
                prod2 = prods.tile([P, 4, VROWS], F32, tag="prodop")
                nc.vector.tensor_tensor(
                    prod2[:], opr[:], out_b[:, None, :].to_broadcast((P, 4, VROWS)),
                    OP.mult)
                for a in range(4):
                    nc.scalar.activation(
                        out=prod2[:, a, :], in_=prod2[:, a, :], func=AF.Copy,
                        accum_out=hid_cols[:, 4 * td + a:4 * td + a + 1])
            nc.sync.dma_start(out=hid_cols_out[:, :], in_=hid_cols[:])

    _split_multi_waits(nc)
    return nc


def _shard_inputs(inputs: dict) -> list[dict]:
    """Slice the full inputs into 8 per-core input maps (kernel layouts)."""
    hidden_in = np.ascontiguousarray(inputs["hidden_in"], dtype=np.float32)
    conv_state = np.ascontiguousarray(inputs["conv_state"], dtype=np.float32)
    rec_state = np.ascontiguousarray(inputs["rec_state"], dtype=np.float32)
    conv_w = np.ascontiguousarray(inputs["conv_w"], dtype=np.float32)
    qkv_w = np.ascontiguousarray(inputs["in_proj_qkv_w"], dtype=np.float32)
    z_w = np.ascontiguousarray(inputs["in_proj_z_w"], dtype=np.float32)
    b_w = np.ascontiguousarray(inputs["in_proj_b_w"], dtype=np.float32)
    a_w = np.ascontiguousarray(inputs["in_proj_a_w"], dtype=np.float32)
    op_w = np.ascontiguousarray(inputs["out_proj_w"], dtype=np.float32)
    dt_bias = np.ascontiguousarray(inputs["dt_bias"], dtype=np.float32)
    a_log = np.ascontiguousarray(inputs["A_log"], dtype=np.float32)
    norm_w = np.ascontiguousarray(inputs["norm_w"], dtype=np.float32)

    hvec = hidden_in.reshape(H)
    in_maps = []
    for i in range(NCORES):
        qsl = slice(QROWS * i, QROWS * (i + 1))
        ksl = slice(KEY_DIM + QROWS * i, KEY_DIM + QROWS * (i + 1))
        vsl = slice(2 * KEY_DIM + VROWS * i, 2 * KEY_DIM + VROWS * (i + 1))
        hsl = slice(NVH * i, NVH * (i + 1))
        zsl = slice(VROWS * i, VROWS * (i + 1))

        def crows(x):
            return np.concatenate([x[qsl], x[ksl], x[vsl]], axis=0)

        w_zba = np.concatenate(
            [z_w[zsl], b_w[hsl], a_w[hsl]], axis=0)  # (520, H)
        cst = crows(conv_state[0]).reshape(NCT, P, KSZ).transpose(1, 0, 2)
        cwt = crows(conv_w).reshape(NCT, P, KSZ).transpose(1, 0, 2)
        rec_i = rec_state[0, hsl].transpose(1, 0, 2)  # (k, h, v)

        in_maps.append({
            "hvec": np.ascontiguousarray(hvec),
            "qkv_w": np.ascontiguousarray(crows(qkv_w)),
            "wpe_t": np.ascontiguousarray(w_zba.T),
            "conv_st": np.ascontiguousarray(cst),
            "conv_wt": np.ascontiguousarray(cwt),
            "rec_in": np.ascontiguousarray(rec_i),
            "op_t": np.ascontiguousarray(op_w[:H // 2, zsl].T),
            "op_rows": np.ascontiguousarray(op_w[H // 2:, zsl]),
            "dt_b": np.ascontiguousarray(dt_bias[hsl]),
            "a_log": np.ascontiguousarray(a_log[hsl]),
            "norm_w": np.ascontiguousarray(norm_w),
        })
    return in_maps


def _gather_outputs(results: list[dict]) -> tuple:
    hidden = np.zeros(H, dtype=np.float64)
    conv_full = np.zeros((CONV_DIM, KSZ), dtype=np.float32)
    rec_full = np.zeros((NUM_V, DK, DV), dtype=np.float32)
    for i, r in enumerate(results):
        hidden[:H // 2] += r["hid_row_out"].astype(np.float64)
        hidden[H // 2:] += r["hid_cols_out"].astype(np.float64).T.reshape(H // 2)
        ncs = r["conv_out"].transpose(1, 0, 2).reshape(CLOC, KSZ)
        conv_full[QROWS * i:QROWS * (i + 1)] = ncs[:QROWS]
        conv_full[KEY_DIM + QROWS * i:KEY_DIM + QROWS * (i + 1)] = ncs[QROWS:2 * QROWS]
        conv_full[2 * KEY_DIM + VROWS * i:2 * KEY_DIM + VROWS * (i + 1)] = ncs[2 * QROWS:]
        rec_full[NVH * i:NVH * (i + 1)] = r["rec_out"].transpose(1, 0, 2)
    return (
        hidden.astype(np.float32).reshape(1, 1, H),
        conv_full.reshape(1, CONV_DIM, KSZ),
        rec_full.reshape(1, NUM_V, DK, DV),
    )


def kernel(**inputs):
    global LAST_RESULTS
    import os
    nc = _CACHE.get("nc")
    if nc is None:
        nc = _build_nc()
        _CACHE["nc"] = nc
    in_maps = _shard_inputs(inputs)
    kwargs = {}
    if os.environ.get("KERNEL_TRACE"):
        kwargs = dict(trace=True, tmpdir=os.environ.get("KERNEL_TRACE_DIR"))
    res = run_bass_kernel_spmd(nc, in_maps, core_ids=list(range(NCORES)), **kwargs)
    LAST_RESULTS = res
    return _gather_outputs(res.results)


# revision 38
# speedup vs baseline: 1.0448x; 1.0131x over previous
"""Trainium2 Bass kernel for a gated-delta-rule decode step (B=1, T=1).

Tensor-parallel over 8 NeuronCores by value-head groups:
  - per core: 2 key heads (DK=128), 4 value heads (DV=128)
  - row-shards of in_proj_{qkv,z,b,a}_w / conv_state / conv_w,
    column-shard of out_proj_w, head-shard of rec_state
  - out_proj partials are summed on the host (8 x 2048 floats).

All GEMVs run on the tensor engine: the host ships W^T, the hidden
vector is the stationary operand ([128,1] per K-chunk), and the weights
stream through as the moving operand, accumulating [1, N<=512] PSUM rows
across 16 K-chunks. The weights therefore stream HBM -> SBUF -> PE once,
with fully contiguous DMAs, and the vector/scalar engines stay nearly
idle (conv, gating, per-head delta-rule rows only).
"""

import numpy as np

import bass_rust
import concourse.bass as bass
import concourse.tile as tile
from concourse import mybir
from concourse.bass_utils import run_bass_kernel_spmd
from concourse.vector_clock import ScopedClock


def _patched_drain_and_barrier(self, tick_clock, wait_clock):
    """Tail drain with at most ONE sem wait per instruction.

    The stock TileContext attaches every outstanding semaphore wait to a
    single Drain; this walrus build's CTRL encoding only has room for one
    sync-wait, so codegen dies with "Too many sync wait commands". Split
    the extra waits onto dedicated NOPs on the same (SP) engine.
    """
    nc = self.nc
    drain_inst = nc.sync.drain()
    wait_clock.add_sem_waits(
        drain_inst.ins, ScopedClock({None: tick_clock.global_clock})
    )
    si = drain_inst.ins.sync_info
    if si is not None and si.on_wait is not None and len(si.on_wait) > 1:
        waits = list(si.on_wait)
        si.on_wait = waits[:1]
        for w in waits[1:]:
            nop = nc.sync.nop(nofuse=True)
            nop.ins.sync_info = bass_rust.SyncInfo(on_wait=[w], on_update=[])
    nc.all_engine_barrier()
    assert self.sems is not None
    popped = nc._tile_sem_poison_stack.pop()
    assert popped is self._sem_poison
    _clear_and_free_semaphores_no_rangeclear(
        nc, list(self.sems.allocated().values()))
    nc.all_engine_barrier()


def _clear_and_free_semaphores_no_rangeclear(nc, sems):
    """clear_and_free_semaphores minus EVENT_SEMAPHORE_RANGE_CLEAR.

    That raw-ISA opcode's 64B encoding is rejected by this walrus build
    ("ISA wrong length"); zero each semaphore with a plain EventSemaphore
    sem-wr-imm update instead, spread across engines.
    """
    if not sems:
        return
    sem_nums = [s.num if isinstance(s, bass.SemaphoreHandle) else s for s in sems]
    for sem_range in bass.compact_to_ranges(sem_nums):
        assert nc._state.free_isdisjoint(sem_range)
        nc.gpsimd.dma_reset(sem_range)
    engines = [nc.gpsimd, nc.vector, nc.scalar, nc.tensor, nc.sync]
    for i, s in enumerate(sems):
        eng = engines[i % len(engines)]
        ev = eng.sem_inc(s, 1)
        u = ev.ins.sync_info.on_update[0]
        ev.ins.sync_info = bass_rust.SyncInfo(
            on_wait=[],
            on_update=[bass_rust.SyncUpdate(
                sync_type='semaphore', id=u.id, ant_name=u.ant_name,
                update_mode='sem-wr-imm', update_value=0, update_reg=None)],
        )
    nc._state.prepend_free_semaphores(sem_nums)
    for poison_set in nc._tile_sem_poison_stack:
        poison_set.update(sem_nums)


tile.TileContext._drain_and_barrier = _patched_drain_and_barrier


def _split_multi_waits(nc):
    """Hoist extra semaphore waits onto same-engine NOPs.

    This walrus build's codegen accepts at most ONE sync-wait per
    instruction ("Too many sync wait commands"); Tile's wait-assignment
    attaches up to three. A NOP on the same engine immediately before the
    instruction is semantically identical (engines execute in order).
    """
    n = 0
    for func in nc.m.functions:
        for blk in func.blocks:
            out = []
            changed = False
            for ins in blk.instructions:
                si = ins.sync_info
                if si is not None and si.on_wait is not None and len(si.on_wait) > 1:
                    waits = list(si.on_wait)
                    for w in waits[:-1]:
                        n += 1
                        nop = mybir.InstNoOp(
                            name=f"WSPLIT-{n}", engine=ins.engine,
                            bass_nofuse=True,
                            sync_info=bass_rust.SyncInfo(on_wait=[w], on_update=[]),
                        )
                        nc.register_instruction(nop, overwrite=True)
                        out.append(nop)
                    si.on_wait = waits[-1:]   # in place: keep fake updates
                    changed = True
                out.append(ins)
            if changed:
                blk.instructions = out


F32 = mybir.dt.float32
AF = mybir.ActivationFunctionType
OP = mybir.AluOpType

# ---- problem constants (hardcoded; kernel.py must be self-contained) ----
H = 2048
NUM_K = 16
NUM_V = 32
DK = 128
DV = 128
KSZ = 4
KEY_DIM = NUM_K * DK            # 2048
VALUE_DIM = NUM_V * DV          # 4096
CONV_DIM = 2 * KEY_DIM + VALUE_DIM  # 8192
EPS = 1e-6
NCORES = 8

# per-core shard sizes
NKH = NUM_K // NCORES           # 2 key heads / core
NVH = NUM_V // NCORES           # 4 value heads / core
QROWS = NKH * DK                # 256
VROWS = NVH * DV                # 512
CLOC = 2 * QROWS + VROWS        # 1024 local conv rows
NCT = CLOC // 128               # 8 local conv tiles
CTOT = CLOC + VROWS + 2 * NVH   # 1544 = qkv + z + b + a
NCH = H // 128                  # 16 K-chunks of the hidden dim
P = 128

_CACHE: dict = {}
LAST_RESULTS = None


def _build_nc(sim_compat: bool = False) -> bass.Bass:
    nc = bass.Bass(trn_type="TRN2")

    # ---- per-core DRAM tensors (kernel-friendly layouts; host permutes) ----
    hvec = nc.dram_tensor("hvec", [H], F32, kind="ExternalInput")
    # qkv rows for the DVE+ACT GEMV path: (1024, H)
    qkv_w = nc.dram_tensor("qkv_w", [CLOC, H], F32, kind="ExternalInput")
    # W^T of [z_w; b_w; a_w] for the PE GEMV path -> (H, 520)
    wpe_t = nc.dram_tensor("wpe_t", [H, VROWS + 2 * NVH], F32,
                           kind="ExternalInput")
    # conv state/weights pre-permuted on host to [p, t, w] (c = t*128 + p)
    conv_st = nc.dram_tensor("conv_st", [P, NCT, KSZ], F32, kind="ExternalInput")
    conv_wt = nc.dram_tensor("conv_wt", [P, NCT, KSZ], F32, kind="ExternalInput")
    # rec pre-permuted on host to [k, h, v]
    rec_in = nc.dram_tensor("rec_in", [DK, NVH, DV], F32, kind="ExternalInput")
    # out_proj column-shard, row layout for the DVE+ACT path: (2048, 512)
    op_rows = nc.dram_tensor("op_rows", [H, VROWS], F32, kind="ExternalInput")
    dt_b = nc.dram_tensor("dt_b", [NVH], F32, kind="ExternalInput")
    a_log = nc.dram_tensor("a_log", [NVH], F32, kind="ExternalInput")
    norm_w = nc.dram_tensor("norm_w", [DV], F32, kind="ExternalInput")

    hid_cols_out = nc.dram_tensor("hid_cols_out", [P, H // P], F32,
                                   kind="ExternalOutput")
    conv_out = nc.dram_tensor("conv_out", [P, NCT, KSZ], F32, kind="ExternalOutput")
    rec_out = nc.dram_tensor("rec_out", [DK, NVH, DV], F32, kind="ExternalOutput")

    # DRAM scratch: broadcast g_exp to all partitions (rec_new scaling),
    # plus row->column re-layouts (SBUF->SBUF transposing DMAs don't balance)
    gex_dram = nc.dram_tensor("gex_scratch", [NVH], F32)
    outv_dram = nc.dram_tensor("outv_scratch", [VROWS], F32)

    with tile.TileContext(nc) as tc:
        with (
            tc.tile_pool(name="singles", bufs=1) as singles,
            tc.tile_pool(name="wpool", bufs=2) as wpool,
            tc.tile_pool(name="prods", bufs=2) as prods,
            tc.tile_pool(name="oppool", bufs=2) as oppool,
            tc.tile_pool(name="rows", bufs=4) as rows,
            tc.tile_pool(name="psy", bufs=1, space="PSUM") as psy,
            tc.tile_pool(name="psh", bufs=2, space="PSUM") as psh,
        ):
            # x as columns (PE stationary operand): xcol[p, c] = h[c*128 + p]
            xcol = singles.tile([P, NCH], F32)
            nc.sync.dma_start(out=xcol[:], in_=hvec.rearrange("(c p) -> p c", p=P))
            # x broadcast to all partitions (DVE GEMV operand)
            xb = singles.tile([P, H], F32)
            hvec_b = bass.AP(
                tensor=hvec[:].tensor, offset=hvec[:].offset,
                ap=[[0, P]] + list(hvec[:].ap),
            )
            nc.sync.dma_start(out=xb[:], in_=hvec_b)

            # ---- z/b/a projection on the PE (psum rows) ----
            ZBA = VROWS + 2 * NVH
            pz = psy.tile([1, VROWS], F32, tag="pz")
            pba = psy.tile([1, 2 * NVH], F32, tag="pba")
            for kd in range(NCH // 4):
                wt2 = wpool.tile([P, 4, ZBA], F32, tag="wpe")
                nc.sync.dma_start(
                    out=wt2[:],
                    in_=wpe_t[512 * kd:512 * (kd + 1), :].rearrange(
                        "(a p) c -> p a c", p=P),
                )
                for a in range(4):
                    kk = 4 * kd + a
                    st = dict(start=(kk == 0), stop=(kk == NCH - 1))
                    lhs = xcol[:, kk:kk + 1]
                    nc.tensor.matmul(pz[:], lhs, wt2[:, a, 0:VROWS], **st)
                    nc.tensor.matmul(pba[:], lhs, wt2[:, a, VROWS:ZBA], **st)

            # ---- qkv projection on DVE (multiply) + ACT (free-axis reduce) --
            # accumulators land directly as conv columns: qcols[p, t]
            qcols = singles.tile([P, NCT], F32)
            for td in range(NCT // 2):
                wt = wpool.tile([P, 2, H], F32, tag="w")
                nc.sync.dma_start(
                    out=wt[:],
                    in_=qkv_w[256 * td:256 * (td + 1), :].rearrange(
                        "(a p) c -> p a c", p=P),
                )
                prod = prods.tile([P, 2, H], F32, tag="prod")
                nc.vector.tensor_tensor(
                    prod[:], wt[:], xb[:, None, :].to_broadcast((P, 2, H)),
                    OP.mult)
                for a in range(2):
                    nc.scalar.activation(
                        out=prod[:, a, :], in_=prod[:, a, :], func=AF.Copy,
                        accum_out=qcols[:, 2 * td + a:2 * td + a + 1])

            # drain psums to SBUF rows
            zba_row = singles.tile([1, ZBA], F32)
            nc.vector.tensor_copy(out=zba_row[:, 0:VROWS], in_=pz[:])
            nc.vector.tensor_copy(out=zba_row[:, VROWS:ZBA], in_=pba[:])
            z_row = zba_row[:, 0:VROWS]

            # ---- causal conv1d update ----
            cs = singles.tile([P, NCT, KSZ], F32)
            cw = singles.tile([P, NCT, KSZ], F32)
            nc.sync.dma_start(out=cs[:], in_=conv_st[:, :, :])
            nc.sync.dma_start(out=cw[:], in_=conv_wt[:, :, :])
            ncs = singles.tile([P, NCT, KSZ], F32)
            nc.vector.tensor_copy(out=ncs[:, :, 0:KSZ - 1], in_=cs[:, :, 1:KSZ])
            nc.vector.tensor_copy(out=ncs[:, :, KSZ - 1:KSZ], in_=qcols[:, :, None])
            nc.sync.dma_start(out=conv_out[:, :, :], in_=ncs[:])

            convp = singles.tile([P, NCT, KSZ], F32)
            nc.vector.tensor_tensor(convp[:], ncs[:], cw[:], OP.mult)
            convpre = singles.tile([P, NCT], F32)
            nc.vector.tensor_reduce(
                out=convpre[:], in_=convp[:], axis=mybir.AxisListType.X, op=OP.add,
            )
            # conv_z_cols columns: 0-1 = k heads, 2-3 = q heads, 4-7 = v heads
            # silu(x) = x / (1 + exp(-x)); only exp/ln ACT tables exist here
            conv_z_cols = singles.tile([P, 32], F32)
            nc.vector.memset(conv_z_cols[:], 0.0)
            sgt = singles.tile([P, NCT], F32)
            nc.scalar.activation(out=sgt[:], in_=convpre[:], func=AF.Exp, scale=-1.0)
            nc.vector.tensor_scalar_add(out=sgt[:], in0=sgt[:], scalar1=1.0)
            nc.vector.reciprocal(out=sgt[:], in_=sgt[:])
            silu_all = singles.tile([P, NCT], F32)
            nc.vector.tensor_tensor(silu_all[:], convpre[:], sgt[:], OP.mult)
            nc.vector.tensor_copy(out=conv_z_cols[:, 2:4], in_=silu_all[:, 0:2])
            nc.vector.tensor_copy(out=conv_z_cols[:, 0:2], in_=silu_all[:, 2:4])
            nc.vector.tensor_copy(out=conv_z_cols[:, 4:NCT], in_=silu_all[:, 4:NCT])

            # ---- transpose -> rows_t[32, 128]; rows 0-1 k, 2-3 q, 4-7 v ----
            rows_t = singles.tile([32, P], F32)
            for b in range(4):
                nc.vector.transpose(
                    out=rows_t[:, 32 * b:32 * (b + 1)],
                    in_=conv_z_cols[32 * b:32 * (b + 1), 0:32],
                )
            # head rows onto partition 0 (engines need base 0/32/64/96)
            rows0 = singles.tile([1, NCT, P], F32)
            nc.sync.dma_start(out=rows0[:], in_=rows_t[0:NCT, None, :])

            # ---- per-head scalars, all on partition 0 ----
            # ss: sum of squares of k/q rows -> ss_row[0, 0:4] (k0,k1,q0,q1)
            sqr = singles.tile([1, 4, P], F32)
            nc.vector.tensor_tensor(sqr[:], rows0[:, 0:4, :], rows0[:, 0:4, :],
                                    OP.mult)
            ss_row = singles.tile([1, 4], F32)
            nc.vector.tensor_reduce(
                out=ss_row[:], in_=sqr[:], axis=mybir.AxisListType.X, op=OP.add)

            eps1 = singles.tile([1, 1], F32)
            nc.vector.memset(eps1[:], EPS)
            epsk1 = singles.tile([1, 1], F32)
            nc.vector.memset(epsk1[:], float(DK) * EPS)
            # rv_row: cols 0-1 rq_eff(g) = 1/sqrt(128*(ss_q+eps)),
            #         cols 2-3 rk(g)     = 1/sqrt(ss_k+eps)
            l_row = singles.tile([1, 4], F32)
            nc.scalar.activation(out=l_row[:, 0:2], in_=ss_row[:, 2:4], func=AF.Ln,
                                 bias=epsk1[:], scale=float(DK))
            nc.scalar.activation(out=l_row[:, 2:4], in_=ss_row[:, 0:2], func=AF.Ln,
                                 bias=eps1[:], scale=1.0)
            rv_row = singles.tile([1, 4], F32)
            nc.scalar.activation(out=rv_row[:], in_=l_row[:], func=AF.Exp,
                                 scale=-0.5)

            # gating from ba row
            b_row = zba_row[:, VROWS:VROWS + NVH]
            a_row = zba_row[:, VROWS + NVH:ZBA]
            dt_row = singles.tile([1, NVH], F32)
            nc.sync.dma_start(out=dt_row[:], in_=dt_b[None, :])
            al_row = singles.tile([1, NVH], F32)
            nc.sync.dma_start(out=al_row[:], in_=a_log[None, :])
            norm_row = singles.tile([1, DV], F32)
            nc.sync.dma_start(out=norm_row[:], in_=norm_w[None, :])

            beta_row = singles.tile([1, NVH], F32)
            nc.scalar.activation(out=beta_row[:], in_=b_row, func=AF.Exp, scale=-1.0)
            nc.vector.tensor_scalar_add(out=beta_row[:], in0=beta_row[:], scalar1=1.0)
            nc.vector.reciprocal(out=beta_row[:], in_=beta_row[:])

            t4a = singles.tile([1, NVH], F32)
            nc.vector.tensor_tensor(t4a[:], a_row, dt_row[:], OP.add)
            sp4 = singles.tile([1, NVH], F32)
            nc.scalar.activation(out=sp4[:], in_=t4a[:], func=AF.Exp)
            nc.scalar.activation(out=sp4[:], in_=sp4[:], func=AF.Ln, bias=1.0)
            ea4 = singles.tile([1, NVH], F32)
            nc.scalar.activation(out=ea4[:], in_=al_row[:], func=AF.Exp)
            t4b = singles.tile([1, NVH], F32)
            nc.vector.tensor_tensor(t4b[:], ea4[:], sp4[:], OP.mult)
            gexp_row = singles.tile([1, NVH], F32)
            nc.scalar.activation(out=gexp_row[:], in_=t4b[:], func=AF.Exp, scale=-1.0)

            # broadcast g_exp to all partitions via DRAM (for rec_new scaling)
            nc.sync.dma_start(out=gex_dram[:], in_=gexp_row[:])
            gexp_b = singles.tile([P, NVH], F32)
            gex_src = bass.AP(
                tensor=gex_dram[:].tensor, offset=gex_dram[:].offset,
                ap=[[0, P]] + list(gex_dram[:].ap),
            )
            nc.sync.dma_start(out=gexp_b[:], in_=gex_src)

            # cg = g_exp * rk(g); nbr = -beta * rk(g)   (per head, partition 0)
            rk_rep = rv_row[:, 2:4, None].to_broadcast((1, 2, 2))
            cg_row = singles.tile([1, NVH], F32)
            nc.vector.tensor_tensor(
                cg_row[:].rearrange("o (a b) -> o a b", a=2),
                gexp_row[:].rearrange("o (a b) -> o a b", a=2), rk_rep, OP.mult)
            nbr_row = singles.tile([1, NVH], F32)
            nc.vector.scalar_tensor_tensor(
                out=nbr_row[:].rearrange("o (a b) -> o a b", a=2),
                in0=beta_row[:].rearrange("o (a b) -> o a b", a=2),
                scalar=-1.0, in1=rk_rep, op0=OP.mult, op1=OP.mult)

            # ---- recurrent state update per head ----
            rec_sb = singles.tile([DK, NVH, DV], F32)
            nc.sync.dma_start(out=rec_sb[:], in_=rec_in[:, :, :])
            rec_new = singles.tile([DK, NVH, DV], F32)
            out_cat = singles.tile([1, VROWS], F32)

            for h in range(NVH):
                g = h // 2
                qcol = conv_z_cols[:, 2 + g:3 + g]
                kcol = conv_z_cols[:, g:g + 1]
                krow = rows0[:, g, :]
                vrow = rows0[:, 4 + h, :]
                zrow = z_row[:, h * DV:(h + 1) * DV]
                rec_h = rec_sb[:, h, :]

                # m = rec . k_raw
                psum_m = psh.tile([1, DV], F32, tag="pm")
                nc.tensor.matmul(psum_m[:], kcol, rec_h, start=True, stop=True)
                # delta_eff = ((g_exp*rk)*m - v) * (-beta*rk)
                delta1 = rows.tile([1, DV], F32, tag="d1")
                nc.vector.scalar_tensor_tensor(
                    out=delta1[:], in0=psum_m[:], scalar=cg_row[:, h:h + 1],
                    in1=vrow, op0=OP.mult, op1=OP.subtract)
                delta_eff = rows.tile([1, DV], F32, tag="de")
                nc.vector.tensor_scalar_mul(
                    out=delta_eff[:], in0=delta1[:], scalar1=nbr_row[:, h:h + 1])
                # rank-1 update + decay
                psum_rec = psh.tile([DK, DV], F32, tag="pr")
                nc.tensor.matmul(psum_rec[:], krow, delta_eff[:], start=True,
                                 stop=True)
                nc.vector.scalar_tensor_tensor(
                    out=rec_new[:, h, :], in0=rec_h, scalar=gexp_b[:, h:h + 1],
                    in1=psum_rec[:], op0=OP.mult, op1=OP.add)
                # core = rq_eff * (rec_new . q_raw)
                psum_c = psh.tile([1, DV], F32, tag="pc")
                nc.tensor.matmul(psum_c[:], qcol, rec_new[:, h, :], start=True,
                                 stop=True)
                core_row = rows.tile([1, DV], F32, tag="cr")
                nc.vector.tensor_scalar_mul(
                    out=core_row[:], in0=psum_c[:], scalar1=rv_row[:, g:g + 1])

                # RMS norm + silu(z) gate
                sq1 = rows.tile([1, DV], F32, tag="sq1")
                nc.vector.tensor_tensor(sq1[:], core_row[:], core_row[:], OP.mult)
                var1 = rows.tile([1, 1], F32, tag="var")
                nc.vector.tensor_reduce(
                    out=var1[:], in_=sq1[:], axis=mybir.AxisListType.X, op=OP.add)
                sd1 = rows.tile([1, 1], F32, tag="sd")
                nc.scalar.activation(out=sd1[:], in_=var1[:], func=AF.Ln,
                                     bias=eps1[:], scale=1.0 / DV)
                rstd = rows.tile([1, 1], F32, tag="rstd")
                nc.scalar.activation(out=rstd[:], in_=sd1[:], func=AF.Exp,
                                     scale=-0.5)
                siluz = rows.tile([1, DV], F32, tag="sz")
                nc.scalar.activation(out=siluz[:], in_=zrow, func=AF.Exp, scale=-1.0)
                nc.vector.tensor_scalar_add(out=siluz[:], in0=siluz[:], scalar1=1.0)
                nc.vector.reciprocal(out=siluz[:], in_=siluz[:])
                nc.vector.tensor_tensor(siluz[:], siluz[:], zrow, OP.mult)
                xn = rows.tile([1, DV], F32, tag="xn")
                nc.vector.scalar_tensor_tensor(
                    out=xn[:], in0=core_row[:], scalar=rstd[:],
                    in1=norm_row[:], op0=OP.mult, op1=OP.mult)
                nc.vector.tensor_tensor(
                    out_cat[:, h * DV:(h + 1) * DV], xn[:], siluz[:], OP.mult)

            nc.sync.dma_start(out=rec_out[:, :, :], in_=rec_new[:])

            nc.sync.dma_start(out=outv_dram[:], in_=out_cat[:])

            # out broadcast to all partitions (DVE op-proj operand)
            out_b = singles.tile([P, VROWS], F32)
            outv_b = bass.AP(
                tensor=outv_dram[:].tensor, offset=outv_dram[:].offset,
                ap=[[0, P]] + list(outv_dram[:].ap),
            )
            nc.sync.dma_start(out=out_b[:], in_=outv_b)

            # ---- out_proj on DVE+ACT (column accumulators) ----
            hid_cols = singles.tile([P, H // P], F32)
            for td in range(4):
                opr = oppool.tile([P, 4, VROWS], F32, tag="oprow")
                nc.sync.dma_start(
                    out=opr[:],
                    in_=op_rows[512 * td:512 * (td + 1), :].rearrange(
                        "(a p) c -> p a c", p=P))
                prod2 = prods.tile([P, 4, VROWS], F32, tag="prodop")
                nc.vector.tensor_tensor(
                    prod2[:], opr[:], out_b[:, None, :].to_broadcast((P, 4, VROWS)),
                    OP.mult)
                for a in range(4):
                    nc.scalar.activation(
                        out=prod2[:, a, :], in_=prod2[:, a, :], func=AF.Copy,
                        accum_out=hid_cols[:, 4 * td + a:4 * td + a + 1])
            nc.sync.dma_start(out=hid_cols_out[:, :], in_=hid_cols[:])

    _split_multi_waits(nc)
    return nc


def _shard_inputs(inputs: dict) -> list[dict]:
    """Slice the full inputs into 8 per-core input maps (kernel layouts)."""
    hidden_in = np.ascontiguousarray(inputs["hidden_in"], dtype=np.float32)
    conv_state = np.ascontiguousarray(inputs["conv_state"], dtype=np.float32)
    rec_state = np.ascontiguousarray(inputs["rec_state"], dtype=np.float32)
    conv_w = np.ascontiguousarray(inputs["conv_w"], dtype=np.float32)
    qkv_w = np.ascontiguousarray(inputs["in_proj_qkv_w"], dtype=np.float32)
    z_w = np.ascontiguousarray(inputs["in_proj_z_w"], dtype=np.float32)
    b_w = np.ascontiguousarray(inputs["in_proj_b_w"], dtype=np.float32)
    a_w = np.ascontiguousarray(inputs["in_proj_a_w"], dtype=np.float32)
    op_w = np.ascontiguousarray(inputs["out_proj_w"], dtype=np.float32)
    dt_bias = np.ascontiguousarray(inputs["dt_bias"], dtype=np.float32)
    a_log = np.ascontiguousarray(inputs["A_log"], dtype=np.float32)
    norm_w = np.ascontiguousarray(inputs["norm_w"], dtype=np.float32)

    hvec = hidden_in.reshape(H)
    in_maps = []
    for i in range(NCORES):
        qsl = slice(QROWS * i, QROWS * (i + 1))
        ksl = slice(KEY_DIM + QROWS * i, KEY_DIM + QROWS * (i + 1))
        vsl = slice(2 * KEY_DIM + VROWS * i, 2 * KEY_DIM + VROWS * (i + 1))
        hsl = slice(NVH * i, NVH * (i + 1))
        zsl = slice(VROWS * i, VROWS * (i + 1))

        def crows(x):
            return np.concatenate([x[qsl], x[ksl], x[vsl]], axis=0)

        w_zba = np.concatenate(
            [z_w[zsl], b_w[hsl], a_w[hsl]], axis=0)  # (520, H)
        cst = crows(conv_state[0]).reshape(NCT, P, KSZ).transpose(1, 0, 2)
        cwt = crows(conv_w).reshape(NCT, P, KSZ).transpose(1, 0, 2)
        rec_i = rec_state[0, hsl].transpose(1, 0, 2)  # (k, h, v)

        in_maps.append({
            "hvec": np.ascontiguousarray(hvec),
            "qkv_w": np.ascontiguousarray(crows(qkv_w)),
            "wpe_t": np.ascontiguousarray(w_zba.T),
            "conv_st": np.ascontiguousarray(cst),
            "conv_wt": np.ascontiguousarray(cwt),
            "rec_in": np.ascontiguousarray(rec_i),
            "op_rows": np.ascontiguousarray(op_w[:, zsl]),
            "dt_b": np.ascontiguousarray(dt_bias[hsl]),
            "a_log": np.ascontiguousarray(a_log[hsl]),
            "norm_w": np.ascontiguousarray(norm_w),
        })
    return in_maps


def _gather_outputs(results: list[dict]) -> tuple:
    hidden = np.zeros(H, dtype=np.float64)
    conv_full = np.zeros((CONV_DIM, KSZ), dtype=np.float32)
    rec_full = np.zeros((NUM_V, DK, DV), dtype=np.float32)
    for i, r in enumerate(results):
        hidden += r["hid_cols_out"].astype(np.float64).T.reshape(H)
        ncs = r["conv_out"].transpose(1, 0, 2).reshape(CLOC, KSZ)
        conv_full[QROWS * i:QROWS * (i + 1)] = ncs[:QROWS]
        conv_full[KEY_DIM + QROWS * i:KEY_DIM + QROWS * (i + 1)] = ncs[QROWS:2 * QROWS]
        conv_full[2 * KEY_DIM + VROWS * i:2 * KEY_DIM + VROWS * (i + 1)] = ncs[2 * QROWS:]
        rec_full[NVH * i:NVH * (i + 1)] = r["rec_out"].transpose(1, 0, 2)
    return (
        hidden.astype(np.float32).reshape(1, 1, H),
        conv_full.reshape(1, CONV_DIM, KSZ),
        rec_full.reshape(1, NUM_V, DK, DV),
    )


def kernel(**inputs):
    global LAST_RESULTS
    import os
    nc = _CACHE.get("nc")
    if nc is None:
        nc = _build_nc()
        _CACHE["nc"] = nc
    in_maps = _shard_inputs(inputs)
    kwargs = {}
    if os.environ.get("KERNEL_TRACE"):
        kwargs = dict(trace=True, tmpdir=os.environ.get("KERNEL_TRACE_DIR"))
    res = run_bass_kernel_spmd(nc, in_maps, core_ids=list(range(NCORES)), **kwargs)
    LAST_RESULTS = res
    return _gather_outputs(res.results)
